# revision 54
# baseline (speedup 1.0000x reference)
"""Trainium2 Bass kernel for nn_Net_76562087018570.

Computation (reference): per-column MinMax scale of a (4096, 8192) f32 matrix,
10 iterations of arr = arr*(1 - (arr - rowmean(arr))) (+0.001 on iter 0),
then inverse transform.

Strategy: shard rows across 8 cores (512 rows each, 4 x (128, 8192) tiles).
State carried as sq_k = (arr_k - h_k)^2 so each iteration is one fused
shift+square+row-sum pass, split between ACT (CA f32 cols, Square activation
with per-row bias and fused row-sum, in place in the load tile) and DVE
(CD f16 cols: TS-sub + TT-square + TS-accum, running at the 2x/4x DVE perf
modes). Startup ((a-mn)*rinv) and final (mn + safe*(gam-sq)) are plain
broadcast passes on DVE; the final result lands in the dead load-tile space
and DMAs out without staging copies. Column min/max: both combine chains on
DVE per 2048-quarter as tiles arrive, then gpsimd partition_all_reduce (max
side) + PE transposes/DVE reduces (min side), one packed AllReduce(max) on
[max | -min] (64 KiB). Broadcast vectors live in the minmax quarter slots.
"""

import os
import numpy as np

R = 512          # rows per core
N = 8192         # columns
NT = 4           # (128,N) row tiles per core
NQ = 4           # 2048-wide column quarters
QW = N // NQ
NCORES = 8
NITERS = 10
CA = 7424        # ACT slice (f32)
CD = N - CA      # 768, DVE slice (f16)

_cache = {}
LAST_RESULT = None


def _build():
    import concourse.bacc as bacc
    import concourse.tile as tile
    from concourse import mybir, masks, bass_isa

    f32 = mybir.dt.float32
    f16 = mybir.dt.float16
    A = mybir.AluOpType
    AF = mybir.ActivationFunctionType
    AX = mybir.AxisListType

    nc = bacc.Bacc(trn_type="TRN2", num_devices=NCORES)
    xs = nc.dram_tensor("xs", [R, N], f32, kind="ExternalInput")
    out = nc.dram_tensor("out", [R, N], f32, kind="ExternalOutput")
    xv = xs.ap().rearrange("(t p) n -> t p n", p=128)
    ov = out.ap().rearrange("(t p) n -> t p n", p=128)

    with tile.TileContext(nc) as tc:
        with tc.tile_pool(name="raw", bufs=1) as raw, \
             tc.tile_pool(name="mmq", bufs=1) as mmq, \
             tc.tile_pool(name="fd", bufs=1) as fd, \
             tc.tile_pool(name="small", bufs=1) as small, \
             tc.tile_pool(name="psumT", bufs=2, space="PSUM") as psumT, \
             tc.tile_pool(name="dram", bufs=1, space="DRAM") as dram:

            ident = small.tile([128, 128], f32)
            masks.make_identity(nc, ident[:])

            # ---- load: row tile split at CA into rawL (f32, persists as the
            # ACT-slice state and output buffer) and rawH ----
            rawL = [raw.tile([128, CA], f32, name=f"aL{t}") for t in range(NT)]
            rawH = [raw.tile([128, CD], f32, name=f"aH{t}")
                    for t in range(NT)]
            for t in range(NT):
                nc.sync.dma_start(rawL[t][:], xv[t][:, 0:CA])
                nc.sync.dma_start(rawH[t][:], xv[t][:, CA:N])

            def rsegs(lo, hi):
                """Split [lo,hi) at CA between rawL/rawH; yield
                (tiles, tlo, thi, glo)."""
                o = []
                if lo < CA:
                    o.append((rawL, lo, min(hi, CA), lo))
                if hi > CA:
                    l = max(lo, CA)
                    o.append((rawH, l - CA, hi - CA, l))
                return o

            # ---- per-core column min/max per quarter, all on DVE ----
            cmax = [mmq.tile([128, QW], f32, name=f"q{j}") for j in range(NQ)]
            cmin = [mmq.tile([128, QW], f32, name=f"r{j}") for j in range(NQ)]
            for j in range(NQ):
                glo = j * QW
                pieces = rsegs(glo, glo + QW)
                for step in range(3):
                    for tl, tlo, thi, g in pieces:
                        d0, d1 = g - glo, g - glo + (thi - tlo)
                        if step == 0:
                            nc.vector.tensor_tensor(
                                cmax[j][:, d0:d1], tl[0][:, tlo:thi],
                                tl[1][:, tlo:thi], op=A.max)
                            nc.vector.tensor_tensor(
                                cmin[j][:, d0:d1], tl[0][:, tlo:thi],
                                tl[1][:, tlo:thi], op=A.min)
                        else:
                            nc.vector.tensor_tensor(
                                cmax[j][:, d0:d1], cmax[j][:, d0:d1],
                                tl[step + 1][:, tlo:thi], op=A.max)
                            nc.vector.tensor_tensor(
                                cmin[j][:, d0:d1], cmin[j][:, d0:d1],
                                tl[step + 1][:, tlo:thi], op=A.min)

            # min side: PE transposes + DVE min reduces -> partition-major
            # (128,64): rmin[p, j*16+cb] = min of col j*2048 + cb*128 + p.
            # max side: gpsimd partition_all_reduce into the cmin tile
            # (free after its transposes); row 0 = column max.
            rmin = small.tile([128, 64], f32)
            for j in range(NQ):
                for g in range(2):
                    pt = psumT.tile([128, 1024], f32, name="pt", tag="pst")
                    for b in range(8):
                        cb = g * 8 + b
                        nc.tensor.transpose(
                            pt[:, b * 128:(b + 1) * 128],
                            cmin[j][:, cb * 128:(cb + 1) * 128],
                            ident[:])
                    nc.vector.tensor_reduce(
                        out=rmin[:, j * 16 + g * 8:j * 16 + g * 8 + 8],
                        in_=pt[:].rearrange("p (c x) -> p c x", c=8),
                        axis=AX.X, op=A.min)
                nc.gpsimd.partition_all_reduce(
                    cmin[j][:], cmax[j][:], channels=128,
                    reduce_op=bass_isa.ReduceOp.max)
            nrmin = small.tile([128, 64], f32)
            nc.vector.tensor_scalar(out=nrmin[:], in0=rmin[:], scalar1=-1.0,
                                    scalar2=None, op0=A.mult)

            # ---- pack [gmax natural | -min partition-major], AllReduce(max)
            cc_in = dram.tile([2, 8192], f32)
            cc_out = dram.tile([2, 8192], f32, addr_space="Shared")
            for j in range(NQ):
                nc.sync.dma_start(cc_in[0:1, j * QW:(j + 1) * QW],
                                  cmin[j][0:1, :])
            nc.sync.dma_start(
                cc_in[1:2, :].rearrange("o (p f) -> (o p) f", p=128), nrmin[:])
            nc.gpsimd.collective_compute(
                "AllReduce", A.max,
                replica_groups=[list(range(NCORES))],
                ins=[cc_in[:]], outs=[cc_out[:]],
            )

            # ---- post-collective scalar math in partition-major (128,64) ----
            gmr = small.tile([64, 128], f32)
            nc.sync.dma_start(
                gmr[:], cc_out[0:1, :].rearrange("o (f p) -> (o f) p", p=128))
            tg = psumT.tile([128, 64], f32, name="tg", tag="pst")
            nc.tensor.transpose(tg[:], gmr[:], ident[0:64, 0:64])
            gmaxP = small.tile([128, 64], f32)
            nc.scalar.copy(gmaxP[:], tg[:])
            nminP = small.tile([128, 64], f32)
            nc.sync.dma_start(
                nminP[:],
                cc_out[1:2, :].rearrange("o (p f) -> (o p) f", p=128))

            rng = small.tile([128, 64], f32)
            nc.vector.tensor_tensor(rng[:], gmaxP[:], nminP[:], op=A.add)
            eq0 = small.tile([128, 64], f32)
            nc.vector.tensor_scalar(out=eq0[:], in0=rng[:], scalar1=0.0,
                                    scalar2=None, op0=A.is_equal)
            safe = small.tile([128, 64], f32)
            nc.vector.tensor_tensor(safe[:], rng[:], eq0[:], op=A.add)
            rinv = small.tile([128, 64], f32)
            nc.vector.reciprocal(rinv[:], safe[:])
            minP = small.tile([128, 64], f32)
            nc.vector.tensor_scalar(out=minP[:], in0=nminP[:], scalar1=-1.0,
                                    scalar2=None, op0=A.mult)

            # relayout to natural rows via pack + PE transposes
            packa = small.tile([128, 128], f32)
            nc.vector.tensor_copy(packa[:, 0:64], minP[:])
            nc.vector.tensor_copy(packa[:, 64:128], rinv[:])
            ta = psumT.tile([128, 128], f32, name="ta", tag="pst")
            nc.tensor.transpose(ta[:], packa[:], ident[:])
            tas = small.tile([128, 128], f32)
            nc.scalar.copy(tas[:], ta[:])
            tb = psumT.tile([64, 128], f32, name="tb", tag="pst")
            nc.tensor.transpose(tb[:], safe[:], ident[:])
            tbs = small.tile([64, 128], f32)
            nc.scalar.copy(tbs[:], tb[:])

            mn_d = dram.tile([1, N], f32)
            rinv_d = dram.tile([1, N], f32)
            safe_d = dram.tile([1, N], f32)
            nc.sync.dma_start(
                mn_d[:].rearrange("o (f p) -> (o f) p", p=128), tas[0:64, :])
            nc.sync.dma_start(
                rinv_d[:].rearrange("o (f p) -> (o f) p", p=128),
                tas[64:128, :])
            nc.sync.dma_start(
                safe_d[:].rearrange("o (f p) -> (o f) p", p=128), tbs[:])

            # broadcasts: mnb in the cmax slots (lives to the end),
            # rb in the cmin slots (dies after startup; safeb reuses them)
            mnb = [mmq.tile([128, QW], f32, name=f"q{j}") for j in range(NQ)]
            rb = [mmq.tile([128, QW], f32, name=f"r{j}") for j in range(NQ)]
            for j in range(NQ):
                nc.sync.dma_start(
                    mnb[j][:],
                    mn_d[0:1, j * QW:(j + 1) * QW].to_broadcast((128, QW)))
                nc.sync.dma_start(
                    rb[j][:],
                    rinv_d[0:1, j * QW:(j + 1) * QW].to_broadcast((128, QW)))

            # ---- f16 DVE-slice state: tile 0 static, tiles 1-3 reuse the
            # rawH slots of the previous tile (fully read by then) ----
            arrD = [fd.tile([128, CD], f16, name="d0")] + \
                   [raw.tile([128, CD], f16, name=f"aH{t - 1}")
                    for t in range(1, NT)]

            sacc = small.tile([128, 8 * NT], f32)
            s0 = small.tile([128, NT], f32)
            h0 = small.tile([128, NT], f32)

            def startup(t):
                na = 0
                for j in range(NQ):
                    glo = j * QW
                    for tl, tlo, thi, g in rsegs(glo, glo + QW):
                        d0 = g - glo
                        w = thi - tlo
                        nc.vector.tensor_tensor(
                            tl[t][:, tlo:thi], tl[t][:, tlo:thi],
                            mnb[j][:, d0:d0 + w], op=A.subtract)
                for j in range(NQ):
                    glo = j * QW
                    for tl, tlo, thi, g in rsegs(glo, glo + QW):
                        d0 = g - glo
                        w = thi - tlo
                        dst = (rawL[t][:, tlo:thi] if tl is rawL
                               else arrD[t][:, tlo:thi])
                        nc.vector.scalar_tensor_tensor(
                            out=dst, in0=tl[t][:, tlo:thi], scalar=0.0,
                            in1=rb[j][:, d0:d0 + w], op0=A.bypass,
                            op1=A.mult,
                            accum_out=sacc[:, 8 * t + na:8 * t + na + 1])
                        na += 1
                nc.vector.tensor_reduce(
                    out=s0[:, t:t + 1], in_=sacc[:, 8 * t:8 * t + na],
                    axis=AX.X, op=A.add)
                nc.vector.tensor_scalar(
                    out=h0[:, t:t + 1], in0=s0[:, t:t + 1], scalar1=0.5 / N,
                    scalar2=0.5, op0=A.mult, op1=A.add)

            # per-group scalar-chain state (group g = tiles 2g, 2g+1)
            G = [dict() for _ in range(2)]

            def ginit(g):
                st = G[g]
                hv = small.tile([128, 2], f32, name=f"gh{g}")
                nc.vector.tensor_copy(hv[:], h0[:, 2 * g:2 * g + 2])
                nshv = small.tile([128, 2], f32, name=f"gnsh{g}")
                nc.vector.tensor_scalar(out=nshv[:], in0=hv[:], scalar1=-1.0,
                                        scalar2=None, op0=A.mult)
                hhv = small.tile([128, 2], f32, name=f"ghh{g}")
                nc.vector.tensor_tensor(hhv[:], hv[:], hv[:], op=A.mult)
                gamv = small.tile([128, 2], f32, name=f"ggam{g}")
                nc.vector.tensor_scalar(out=gamv[:], in0=hhv[:], scalar1=0.001,
                                        scalar2=None, op0=A.add)
                hgv = small.tile([128, 2], f32, name=f"ghg{g}")
                nc.vector.tensor_scalar(out=hgv[:], in0=gamv[:], scalar1=0.5,
                                        scalar2=0.5, op0=A.mult, op1=A.add)
                st.update(sh=hv, nsh=nshv, gam=gamv, hg=hgv)

            def giter(g, k):
                st = G[g]
                last = k == NITERS - 1
                accA = small.tile([128, 2], f32, name=f"gaccA{g}")
                accD = small.tile([128, 2], f32, name=f"gaccD{g}")
                for i, t in enumerate((2 * g, 2 * g + 1)):
                    nc.scalar.activation(
                        rawL[t][:], rawL[t][:], AF.Square,
                        bias=st["nsh"][:, i:i + 1], scale=1.0,
                        accum_out=accA[:, i:i + 1])
                    adv = arrD[t][:]
                    nc.vector.tensor_scalar(
                        out=adv, in0=adv, scalar1=st["sh"][:, i:i + 1],
                        scalar2=None, op0=A.subtract)
                    if last:
                        nc.vector.tensor_tensor(adv, adv, adv, op=A.mult)
                    else:
                        nc.vector.scalar_tensor_tensor(
                            out=adv, in0=adv, scalar=0.0, in1=adv,
                            op0=A.bypass, op1=A.mult,
                            accum_out=accD[:, i:i + 1])
                if last:
                    return
                ss = small.tile([128, 2], f32, name=f"gss{g}")
                nc.vector.tensor_tensor(ss[:], accA[:], accD[:], op=A.add)
                hv = small.tile([128, 2], f32, name=f"gh{g}")
                nc.vector.scalar_tensor_tensor(
                    out=hv[:], in0=ss[:], scalar=-0.5 / N, in1=st["hg"][:],
                    op0=A.mult, op1=A.add)
                beta = small.tile([128, 2], f32, name=f"gbeta{g}")
                nc.vector.tensor_tensor(beta[:], st["gam"][:], hv[:],
                                        op=A.subtract)
                nshv = small.tile([128, 2], f32, name=f"gnsh{g}")
                nc.vector.tensor_scalar(out=nshv[:], in0=beta[:], scalar1=-1.0,
                                        scalar2=None, op0=A.mult)
                gamv = small.tile([128, 2], f32, name=f"ggam{g}")
                nc.vector.tensor_tensor(gamv[:], hv[:], hv[:], op=A.mult)
                hgv = small.tile([128, 2], f32, name=f"ghg{g}")
                nc.vector.tensor_scalar(out=hgv[:], in0=gamv[:], scalar1=0.5,
                                        scalar2=0.5, op0=A.mult, op1=A.add)
                st.update(sh=beta, nsh=nshv, gam=gamv, hg=hgv)

            # safeb broadcast reuses the rb slots; issue before the pipeline
            # tail needs it (rb is fully read once all startups are issued,
            # so emit after the last startup in the schedule below).
            def bc_safeb():
                sb = [mmq.tile([128, QW], f32, name=f"r{j}")
                      for j in range(NQ)]
                for j in range(NQ):
                    nc.sync.dma_start(
                        sb[j][:],
                        safe_d[0:1, j * QW:(j + 1) * QW].to_broadcast(
                            (128, QW)))
                return sb

            def final(t):
                g = t // 2
                i = t % 2
                gamv = G[g]["gam"]
                outD = raw.tile([128, CD], f32, name="aH3")
                for j in range(NQ):
                    glo = j * QW
                    for tl, tlo, thi, gg in rsegs(glo, glo + QW):
                        d0 = gg - glo
                        w = thi - tlo
                        src = (rawL[t][:, tlo:thi] if tl is rawL
                               else arrD[t][:, tlo:thi])
                        nc.vector.scalar_tensor_tensor(
                            out=src, in0=src, scalar=gamv[:, i:i + 1],
                            in1=safeb[j][:, d0:d0 + w],
                            op0=A.subtract, op1=A.mult)
                for j in range(NQ):
                    glo = j * QW
                    for tl, tlo, thi, gg in rsegs(glo, glo + QW):
                        d0 = gg - glo
                        w = thi - tlo
                        src = (rawL[t][:, tlo:thi] if tl is rawL
                               else arrD[t][:, tlo:thi])
                        dst = (rawL[t][:, tlo:thi] if tl is rawL
                               else outD[:, tlo:thi])
                        nc.vector.tensor_tensor(
                            dst, mnb[j][:, d0:d0 + w], src, op=A.subtract)
                nc.sync.dma_start(ov[t][:, 0:CA], rawL[t][:])
                nc.sync.dma_start(ov[t][:, CA:N], outD[:])

            # ---- software pipeline: G0 iterates while G1 starts up ----
            startup(0)
            startup(1)
            ginit(0)
            giter(0, 0)
            startup(2)
            giter(0, 1)
            startup(3)
            safeb = bc_safeb()
            giter(0, 2)
            ginit(1)
            for k in range(3, NITERS):
                giter(0, k)
                giter(1, k - 3)
            final(0)
            giter(1, 7)
            final(1)
            giter(1, 8)
            giter(1, 9)
            final(2)
            final(3)

    if not nc.is_finalized():
        nc.finalize()
    return nc


def _get_nc():
    if "nc" not in _cache:
        _cache["nc"] = _build()
    return _cache["nc"]


def kernel(x):
    global LAST_RESULT
    from concourse.bass_utils import run_bass_kernel_spmd

    x = np.ascontiguousarray(np.asarray(x), dtype=np.float32)
    a = x.reshape(NCORES * R, N)
    nc = _get_nc()
    in_maps = [{"xs": np.ascontiguousarray(a[c * R:(c + 1) * R])}
               for c in range(NCORES)]
    res = run_bass_kernel_spmd(
        nc, in_maps, core_ids=list(range(NCORES)),
        trace=bool(int(os.environ.get("KBENCH_TRACE", "0"))),
    )
    LAST_RESULT = res
    full = np.concatenate([res.results[c]["out"] for c in range(NCORES)], axis=0)
    return full.reshape(1, NCORES * R, N).astype(np.float32)


# revision 55
# speedup vs baseline: 1.0700x; 1.0700x over previous
"""Trainium2 Bass kernel for nn_Net_76562087018570.

Computation (reference): per-column MinMax scale of a (4096, 8192) f32 matrix,
10 iterations of arr = arr*(1 - (arr - rowmean(arr))) (+0.001 on iter 0),
then inverse transform.

Strategy: shard rows across 8 cores (512 rows each, 4 x (128, 8192) tiles).
State carried as sq_k = (arr_k - h_k)^2 so each iteration is one fused
shift+square+row-sum pass, split between ACT (CA f32 cols, Square activation
with per-row bias and fused row-sum, in place in the load tile) and DVE
(CD f16 cols: TS-sub + TT-square + TS-accum, running at the 2x/4x DVE perf
modes). Startup ((a-mn)*rinv) and final (mn + safe*(gam-sq)) are plain
broadcast passes on DVE; the final result lands in the dead load-tile space
and DMAs out without staging copies. Column min/max: both combine chains on
DVE per 2048-quarter as tiles arrive, then gpsimd partition_all_reduce (max
side) + PE transposes/DVE reduces (min side), one packed AllReduce(max) on
[max | -min] (64 KiB). Broadcast vectors live in the minmax quarter slots.
"""

import os
import numpy as np

R = 512          # rows per core
N = 8192         # columns
NT = 4           # (128,N) row tiles per core
NQ = 4           # 2048-wide column quarters
QW = N // NQ
NCORES = 8
NITERS = 10
CA = 6272        # ACT slice (f32)
CD = N - CA      # 1920, DVE slice (f16)

_cache = {}
LAST_RESULT = None


def _build():
    import concourse.bacc as bacc
    import concourse.tile as tile
    from concourse import mybir, masks, bass_isa

    f32 = mybir.dt.float32
    f16 = mybir.dt.float16
    A = mybir.AluOpType
    AF = mybir.ActivationFunctionType
    AX = mybir.AxisListType

    nc = bacc.Bacc(trn_type="TRN2", num_devices=NCORES)
    xs = nc.dram_tensor("xs", [R, N], f32, kind="ExternalInput")
    out = nc.dram_tensor("out", [R, N], f32, kind="ExternalOutput")
    xv = xs.ap().rearrange("(t p) n -> t p n", p=128)
    ov = out.ap().rearrange("(t p) n -> t p n", p=128)

    with tile.TileContext(nc) as tc:
        with tc.tile_pool(name="raw", bufs=1) as raw, \
             tc.tile_pool(name="mmq", bufs=1) as mmq, \
             tc.tile_pool(name="fd", bufs=1) as fd, \
             tc.tile_pool(name="small", bufs=1) as small, \
             tc.tile_pool(name="psumT", bufs=2, space="PSUM") as psumT, \
             tc.tile_pool(name="dram", bufs=1, space="DRAM") as dram:

            ident = small.tile([128, 128], f32)
            masks.make_identity(nc, ident[:])

            # ---- load: row tile split at CA into rawL (f32, persists as the
            # ACT-slice state and output buffer) and rawH ----
            rawL = [raw.tile([128, CA], f32, name=f"aL{t}") for t in range(NT)]
            rawH = [raw.tile([128, CD], f32, name=f"aH{t}")
                    for t in range(NT)]
            for t in range(NT):
                nc.sync.dma_start(rawL[t][:], xv[t][:, 0:CA])
                nc.sync.dma_start(rawH[t][:], xv[t][:, CA:N])

            def rsegs(lo, hi):
                """Split [lo,hi) at CA between rawL/rawH; yield
                (tiles, tlo, thi, glo)."""
                o = []
                if lo < CA:
                    o.append((rawL, lo, min(hi, CA), lo))
                if hi > CA:
                    l = max(lo, CA)
                    o.append((rawH, l - CA, hi - CA, l))
                return o

            # ---- per-core column min/max per quarter, all on DVE ----
            cmax = [mmq.tile([128, QW], f32, name=f"q{j}") for j in range(NQ)]
            cmin = [mmq.tile([128, QW], f32, name=f"r{j}") for j in range(NQ)]
            for j in range(NQ):
                glo = j * QW
                pieces = rsegs(glo, glo + QW)
                for step in range(3):
                    for tl, tlo, thi, g in pieces:
                        d0, d1 = g - glo, g - glo + (thi - tlo)
                        if step == 0:
                            nc.vector.tensor_tensor(
                                cmax[j][:, d0:d1], tl[0][:, tlo:thi],
                                tl[1][:, tlo:thi], op=A.max)
                            nc.vector.tensor_tensor(
                                cmin[j][:, d0:d1], tl[0][:, tlo:thi],
                                tl[1][:, tlo:thi], op=A.min)
                        else:
                            nc.vector.tensor_tensor(
                                cmax[j][:, d0:d1], cmax[j][:, d0:d1],
                                tl[step + 1][:, tlo:thi], op=A.max)
                            nc.vector.tensor_tensor(
                                cmin[j][:, d0:d1], cmin[j][:, d0:d1],
                                tl[step + 1][:, tlo:thi], op=A.min)

            # min side: PE transposes + DVE min reduces -> partition-major
            # (128,64): rmin[p, j*16+cb] = min of col j*2048 + cb*128 + p.
            # max side: gpsimd partition_all_reduce into the cmin tile
            # (free after its transposes); row 0 = column max.
            rmin = small.tile([128, 64], f32)
            for j in range(NQ):
                for g in range(2):
                    pt = psumT.tile([128, 1024], f32, name="pt", tag="pst")
                    for b in range(8):
                        cb = g * 8 + b
                        nc.tensor.transpose(
                            pt[:, b * 128:(b + 1) * 128],
                            cmin[j][:, cb * 128:(cb + 1) * 128],
                            ident[:])
                    nc.vector.tensor_reduce(
                        out=rmin[:, j * 16 + g * 8:j * 16 + g * 8 + 8],
                        in_=pt[:].rearrange("p (c x) -> p c x", c=8),
                        axis=AX.X, op=A.min)
                nc.gpsimd.partition_all_reduce(
                    cmin[j][:], cmax[j][:], channels=128,
                    reduce_op=bass_isa.ReduceOp.max)
            nrmin = small.tile([128, 64], f32)
            nc.vector.tensor_scalar(out=nrmin[:], in0=rmin[:], scalar1=-1.0,
                                    scalar2=None, op0=A.mult)

            # ---- pack [gmax natural | -min partition-major], AllReduce(max)
            cc_in = dram.tile([2, 8192], f32)
            cc_out = dram.tile([2, 8192], f32, addr_space="Shared")
            for j in range(NQ):
                nc.sync.dma_start(cc_in[0:1, j * QW:(j + 1) * QW],
                                  cmin[j][0:1, :])
            nc.sync.dma_start(
                cc_in[1:2, :].rearrange("o (p f) -> (o p) f", p=128), nrmin[:])
            nc.gpsimd.collective_compute(
                "AllReduce", A.max,
                replica_groups=[list(range(NCORES))],
                ins=[cc_in[:]], outs=[cc_out[:]],
            )

            # ---- post-collective scalar math in partition-major (128,64) ----
            gmr = small.tile([64, 128], f32)
            nc.sync.dma_start(
                gmr[:], cc_out[0:1, :].rearrange("o (f p) -> (o f) p", p=128))
            tg = psumT.tile([128, 64], f32, name="tg", tag="pst")
            nc.tensor.transpose(tg[:], gmr[:], ident[0:64, 0:64])
            gmaxP = small.tile([128, 64], f32)
            nc.scalar.copy(gmaxP[:], tg[:])
            nminP = small.tile([128, 64], f32)
            nc.sync.dma_start(
                nminP[:],
                cc_out[1:2, :].rearrange("o (p f) -> (o p) f", p=128))

            rng = small.tile([128, 64], f32)
            nc.vector.tensor_tensor(rng[:], gmaxP[:], nminP[:], op=A.add)
            eq0 = small.tile([128, 64], f32)
            nc.vector.tensor_scalar(out=eq0[:], in0=rng[:], scalar1=0.0,
                                    scalar2=None, op0=A.is_equal)
            safe = small.tile([128, 64], f32)
            nc.vector.tensor_tensor(safe[:], rng[:], eq0[:], op=A.add)
            rinv = small.tile([128, 64], f32)
            nc.vector.reciprocal(rinv[:], safe[:])
            minP = small.tile([128, 64], f32)
            nc.vector.tensor_scalar(out=minP[:], in0=nminP[:], scalar1=-1.0,
                                    scalar2=None, op0=A.mult)

            # relayout to natural rows via pack + PE transposes
            packa = small.tile([128, 128], f32)
            nc.vector.tensor_copy(packa[:, 0:64], minP[:])
            nc.vector.tensor_copy(packa[:, 64:128], rinv[:])
            ta = psumT.tile([128, 128], f32, name="ta", tag="pst")
            nc.tensor.transpose(ta[:], packa[:], ident[:])
            tas = small.tile([128, 128], f32)
            nc.scalar.copy(tas[:], ta[:])
            tb = psumT.tile([64, 128], f32, name="tb", tag="pst")
            nc.tensor.transpose(tb[:], safe[:], ident[:])
            tbs = small.tile([64, 128], f32)
            nc.scalar.copy(tbs[:], tb[:])

            mn_d = dram.tile([1, N], f32)
            rinv_d = dram.tile([1, N], f32)
            safe_d = dram.tile([1, N], f32)
            nc.sync.dma_start(
                mn_d[:].rearrange("o (f p) -> (o f) p", p=128), tas[0:64, :])
            nc.sync.dma_start(
                rinv_d[:].rearrange("o (f p) -> (o f) p", p=128),
                tas[64:128, :])
            nc.sync.dma_start(
                safe_d[:].rearrange("o (f p) -> (o f) p", p=128), tbs[:])

            # broadcasts: mnb in the cmax slots (lives to the end),
            # rb in the cmin slots (dies after startup; safeb reuses them)
            mnb = [mmq.tile([128, QW], f32, name=f"q{j}") for j in range(NQ)]
            rb = [mmq.tile([128, QW], f32, name=f"r{j}") for j in range(NQ)]
            for j in range(NQ):
                nc.sync.dma_start(
                    mnb[j][:],
                    mn_d[0:1, j * QW:(j + 1) * QW].to_broadcast((128, QW)))
                nc.sync.dma_start(
                    rb[j][:],
                    rinv_d[0:1, j * QW:(j + 1) * QW].to_broadcast((128, QW)))

            # ---- f16 DVE-slice state: tile 0 static, tiles 1-3 reuse the
            # rawH slots of the previous tile (fully read by then) ----
            arrD = [fd.tile([128, CD], f16, name="d0")] + \
                   [raw.tile([128, CD], f16, name=f"aH{t - 1}")
                    for t in range(1, NT)]

            sacc = small.tile([128, 8 * NT], f32)
            s0 = small.tile([128, NT], f32)
            h0 = small.tile([128, NT], f32)

            def startup(t):
                na = 0
                for j in range(NQ):
                    glo = j * QW
                    for tl, tlo, thi, g in rsegs(glo, glo + QW):
                        d0 = g - glo
                        w = thi - tlo
                        nc.vector.tensor_tensor(
                            tl[t][:, tlo:thi], tl[t][:, tlo:thi],
                            mnb[j][:, d0:d0 + w], op=A.subtract)
                for j in range(NQ):
                    glo = j * QW
                    for tl, tlo, thi, g in rsegs(glo, glo + QW):
                        d0 = g - glo
                        w = thi - tlo
                        dst = (rawL[t][:, tlo:thi] if tl is rawL
                               else arrD[t][:, tlo:thi])
                        nc.vector.scalar_tensor_tensor(
                            out=dst, in0=tl[t][:, tlo:thi], scalar=0.0,
                            in1=rb[j][:, d0:d0 + w], op0=A.bypass,
                            op1=A.mult,
                            accum_out=sacc[:, 8 * t + na:8 * t + na + 1])
                        na += 1
                nc.vector.tensor_reduce(
                    out=s0[:, t:t + 1], in_=sacc[:, 8 * t:8 * t + na],
                    axis=AX.X, op=A.add)
                nc.vector.tensor_scalar(
                    out=h0[:, t:t + 1], in0=s0[:, t:t + 1], scalar1=0.5 / N,
                    scalar2=0.5, op0=A.mult, op1=A.add)

            # per-group scalar-chain state (group g = tiles 2g, 2g+1)
            G = [dict() for _ in range(2)]

            def ginit(g):
                st = G[g]
                hv = small.tile([128, 2], f32, name=f"gh{g}")
                nc.vector.tensor_copy(hv[:], h0[:, 2 * g:2 * g + 2])
                nshv = small.tile([128, 2], f32, name=f"gnsh{g}")
                nc.vector.tensor_scalar(out=nshv[:], in0=hv[:], scalar1=-1.0,
                                        scalar2=None, op0=A.mult)
                hhv = small.tile([128, 2], f32, name=f"ghh{g}")
                nc.vector.tensor_tensor(hhv[:], hv[:], hv[:], op=A.mult)
                gamv = small.tile([128, 2], f32, name=f"ggam{g}")
                nc.vector.tensor_scalar(out=gamv[:], in0=hhv[:], scalar1=0.001,
                                        scalar2=None, op0=A.add)
                hgv = small.tile([128, 2], f32, name=f"ghg{g}")
                nc.vector.tensor_scalar(out=hgv[:], in0=gamv[:], scalar1=0.5,
                                        scalar2=0.5, op0=A.mult, op1=A.add)
                st.update(sh=hv, nsh=nshv, gam=gamv, hg=hgv)

            def giter(g, k):
                st = G[g]
                last = k == NITERS - 1
                accA = small.tile([128, 2], f32, name=f"gaccA{g}")
                accD = small.tile([128, 2], f32, name=f"gaccD{g}")
                for i, t in enumerate((2 * g, 2 * g + 1)):
                    nc.scalar.activation(
                        rawL[t][:], rawL[t][:], AF.Square,
                        bias=st["nsh"][:, i:i + 1], scale=1.0,
                        accum_out=accA[:, i:i + 1])
                    adv = arrD[t][:]
                    nc.vector.tensor_scalar(
                        out=adv, in0=adv, scalar1=st["sh"][:, i:i + 1],
                        scalar2=None, op0=A.subtract)
                    if last:
                        nc.vector.tensor_tensor(adv, adv, adv, op=A.mult)
                    else:
                        nc.vector.scalar_tensor_tensor(
                            out=adv, in0=adv, scalar=0.0, in1=adv,
                            op0=A.bypass, op1=A.mult,
                            accum_out=accD[:, i:i + 1])
                if last:
                    return
                ss = small.tile([128, 2], f32, name=f"gss{g}")
                nc.vector.tensor_tensor(ss[:], accA[:], accD[:], op=A.add)
                hv = small.tile([128, 2], f32, name=f"gh{g}")
                nc.vector.scalar_tensor_tensor(
                    out=hv[:], in0=ss[:], scalar=-0.5 / N, in1=st["hg"][:],
                    op0=A.mult, op1=A.add)
                beta = small.tile([128, 2], f32, name=f"gbeta{g}")
                nc.vector.tensor_tensor(beta[:], st["gam"][:], hv[:],
                                        op=A.subtract)
                nshv = small.tile([128, 2], f32, name=f"gnsh{g}")
                nc.vector.tensor_scalar(out=nshv[:], in0=beta[:], scalar1=-1.0,
                                        scalar2=None, op0=A.mult)
                gamv = small.tile([128, 2], f32, name=f"ggam{g}")
                nc.vector.tensor_tensor(gamv[:], hv[:], hv[:], op=A.mult)
                hgv = small.tile([128, 2], f32, name=f"ghg{g}")
                nc.vector.tensor_scalar(out=hgv[:], in0=gamv[:], scalar1=0.5,
                                        scalar2=0.5, op0=A.mult, op1=A.add)
                st.update(sh=beta, nsh=nshv, gam=gamv, hg=hgv)

            # safeb broadcast reuses the rb slots; issue before the pipeline
            # tail needs it (rb is fully read once all startups are issued,
            # so emit after the last startup in the schedule below).
            def bc_safeb():
                sb = [mmq.tile([128, QW], f32, name=f"r{j}")
                      for j in range(NQ)]
                for j in range(NQ):
                    nc.sync.dma_start(
                        sb[j][:],
                        safe_d[0:1, j * QW:(j + 1) * QW].to_broadcast(
                            (128, QW)))
                return sb

            def final(t):
                g = t // 2
                i = t % 2
                gamv = G[g]["gam"]
                outD = raw.tile([128, CD], f32, name="aH3")
                for j in range(NQ):
                    glo = j * QW
                    for tl, tlo, thi, gg in rsegs(glo, glo + QW):
                        d0 = gg - glo
                        w = thi - tlo
                        src = (rawL[t][:, tlo:thi] if tl is rawL
                               else arrD[t][:, tlo:thi])
                        nc.vector.scalar_tensor_tensor(
                            out=src, in0=src, scalar=gamv[:, i:i + 1],
                            in1=safeb[j][:, d0:d0 + w],
                            op0=A.subtract, op1=A.mult)
                for j in range(NQ):
                    glo = j * QW
                    for tl, tlo, thi, gg in rsegs(glo, glo + QW):
                        d0 = gg - glo
                        w = thi - tlo
                        src = (rawL[t][:, tlo:thi] if tl is rawL
                               else arrD[t][:, tlo:thi])
                        dst = (rawL[t][:, tlo:thi] if tl is rawL
                               else outD[:, tlo:thi])
                        nc.vector.tensor_tensor(
                            dst, mnb[j][:, d0:d0 + w], src, op=A.subtract)
                nc.sync.dma_start(ov[t][:, 0:CA], rawL[t][:])
                nc.sync.dma_start(ov[t][:, CA:N], outD[:])

            # ---- software pipeline: G0 iterates while G1 starts up ----
            startup(0)
            startup(1)
            ginit(0)
            giter(0, 0)
            startup(2)
            giter(0, 1)
            startup(3)
            safeb = bc_safeb()
            giter(0, 2)
            ginit(1)
            for k in range(3, NITERS):
                giter(0, k)
                giter(1, k - 3)
            final(0)
            giter(1, 7)
            final(1)
            giter(1, 8)
            giter(1, 9)
            final(2)
            final(3)

    if not nc.is_finalized():
        nc.finalize()
    return nc


def _get_nc():
    if "nc" not in _cache:
        _cache["nc"] = _build()
    return _cache["nc"]


def kernel(x):
    global LAST_RESULT
    from concourse.bass_utils import run_bass_kernel_spmd

    x = np.ascontiguousarray(np.asarray(x), dtype=np.float32)
    a = x.reshape(NCORES * R, N)
    nc = _get_nc()
    in_maps = [{"xs": np.ascontiguousarray(a[c * R:(c + 1) * R])}
               for c in range(NCORES)]
    res = run_bass_kernel_spmd(
        nc, in_maps, core_ids=list(range(NCORES)),
        trace=bool(int(os.environ.get("KBENCH_TRACE", "0"))),
    )
    LAST_RESULT = res
    full = np.concatenate([res.results[c]["out"] for c in range(NCORES)], axis=0)
    return full.reshape(1, NCORES * R, N).astype(np.float32)


# revision 56
# speedup vs baseline: 1.0781x; 1.0075x over previous
"""Trainium2 Bass kernel for nn_Net_76562087018570.

Computation (reference): per-column MinMax scale of a (4096, 8192) f32 matrix,
10 iterations of arr = arr*(1 - (arr - rowmean(arr))) (+0.001 on iter 0),
then inverse transform.

Strategy: shard rows across 8 cores (512 rows each, 4 x (128, 8192) tiles).
State carried as sq_k = (arr_k - h_k)^2 so each iteration is one fused
shift+square+row-sum pass, split between ACT (CA f32 cols, Square activation
with per-row bias and fused row-sum, in place in the load tile) and DVE
(CD f16 cols: TS-sub + TT-square + TS-accum, running at the 2x/4x DVE perf
modes). Startup ((a-mn)*rinv) and final (mn + safe*(gam-sq)) are plain
broadcast passes on DVE; the final result lands in the dead load-tile space
and DMAs out without staging copies. Column min/max: both combine chains on
DVE per 2048-quarter as tiles arrive, then gpsimd partition_all_reduce (max
side) + PE transposes/DVE reduces (min side), one packed AllReduce(max) on
[max | -min] (64 KiB). Broadcast vectors live in the minmax quarter slots.
"""

import os
import numpy as np

R = 512          # rows per core
N = 8192         # columns
NT = 4           # (128,N) row tiles per core
NQ = 4           # 2048-wide column quarters
QW = N // NQ
NCORES = 8
NITERS = 10
CA = 5888        # ACT slice (f32)
CD = N - CA      # 2304, DVE slice (f16)

_cache = {}
LAST_RESULT = None


def _build():
    import concourse.bacc as bacc
    import concourse.tile as tile
    from concourse import mybir, masks, bass_isa

    f32 = mybir.dt.float32
    f16 = mybir.dt.float16
    A = mybir.AluOpType
    AF = mybir.ActivationFunctionType
    AX = mybir.AxisListType

    nc = bacc.Bacc(trn_type="TRN2", num_devices=NCORES)
    xs = nc.dram_tensor("xs", [R, N], f32, kind="ExternalInput")
    out = nc.dram_tensor("out", [R, N], f32, kind="ExternalOutput")
    xv = xs.ap().rearrange("(t p) n -> t p n", p=128)
    ov = out.ap().rearrange("(t p) n -> t p n", p=128)

    with tile.TileContext(nc) as tc:
        with tc.tile_pool(name="raw", bufs=1) as raw, \
             tc.tile_pool(name="mmq", bufs=1) as mmq, \
             tc.tile_pool(name="fd", bufs=1) as fd, \
             tc.tile_pool(name="small", bufs=1) as small, \
             tc.tile_pool(name="psumT", bufs=2, space="PSUM") as psumT, \
             tc.tile_pool(name="dram", bufs=1, space="DRAM") as dram:

            ident = small.tile([128, 128], f32)
            masks.make_identity(nc, ident[:])

            # ---- load: row tile split at CA into rawL (f32, persists as the
            # ACT-slice state and output buffer) and rawH ----
            rawL = [raw.tile([128, CA], f32, name=f"aL{t}") for t in range(NT)]
            rawH = [raw.tile([128, CD], f32, name=f"aH{t}")
                    for t in range(NT)]
            for t in range(NT):
                nc.sync.dma_start(rawL[t][:], xv[t][:, 0:CA])
                nc.sync.dma_start(rawH[t][:], xv[t][:, CA:N])

            def rsegs(lo, hi):
                """Split [lo,hi) at CA between rawL/rawH; yield
                (tiles, tlo, thi, glo)."""
                o = []
                if lo < CA:
                    o.append((rawL, lo, min(hi, CA), lo))
                if hi > CA:
                    l = max(lo, CA)
                    o.append((rawH, l - CA, hi - CA, l))
                return o

            # ---- per-core column min/max per quarter, all on DVE ----
            cmax = [mmq.tile([128, QW], f32, name=f"q{j}") for j in range(NQ)]
            cmin = [mmq.tile([128, QW], f32, name=f"r{j}") for j in range(NQ)]
            for j in range(NQ):
                glo = j * QW
                pieces = rsegs(glo, glo + QW)
                for step in range(3):
                    for tl, tlo, thi, g in pieces:
                        d0, d1 = g - glo, g - glo + (thi - tlo)
                        if step == 0:
                            nc.vector.tensor_tensor(
                                cmax[j][:, d0:d1], tl[0][:, tlo:thi],
                                tl[1][:, tlo:thi], op=A.max)
                            nc.vector.tensor_tensor(
                                cmin[j][:, d0:d1], tl[0][:, tlo:thi],
                                tl[1][:, tlo:thi], op=A.min)
                        else:
                            nc.vector.tensor_tensor(
                                cmax[j][:, d0:d1], cmax[j][:, d0:d1],
                                tl[step + 1][:, tlo:thi], op=A.max)
                            nc.vector.tensor_tensor(
                                cmin[j][:, d0:d1], cmin[j][:, d0:d1],
                                tl[step + 1][:, tlo:thi], op=A.min)

            # min side: PE transposes + DVE min reduces -> partition-major
            # (128,64): rmin[p, j*16+cb] = min of col j*2048 + cb*128 + p.
            # max side: gpsimd partition_all_reduce into the cmin tile
            # (free after its transposes); row 0 = column max.
            rmin = small.tile([128, 64], f32)
            for j in range(NQ):
                for g in range(2):
                    pt = psumT.tile([128, 1024], f32, name="pt", tag="pst")
                    for b in range(8):
                        cb = g * 8 + b
                        nc.tensor.transpose(
                            pt[:, b * 128:(b + 1) * 128],
                            cmin[j][:, cb * 128:(cb + 1) * 128],
                            ident[:])
                    nc.vector.tensor_reduce(
                        out=rmin[:, j * 16 + g * 8:j * 16 + g * 8 + 8],
                        in_=pt[:].rearrange("p (c x) -> p c x", c=8),
                        axis=AX.X, op=A.min)
                nc.gpsimd.partition_all_reduce(
                    cmin[j][:], cmax[j][:], channels=128,
                    reduce_op=bass_isa.ReduceOp.max)
            nrmin = small.tile([128, 64], f32)
            nc.vector.tensor_scalar(out=nrmin[:], in0=rmin[:], scalar1=-1.0,
                                    scalar2=None, op0=A.mult)

            # ---- pack [gmax natural | -min partition-major], AllReduce(max)
            cc_in = dram.tile([2, 8192], f32)
            cc_out = dram.tile([2, 8192], f32, addr_space="Shared")
            for j in range(NQ):
                nc.sync.dma_start(cc_in[0:1, j * QW:(j + 1) * QW],
                                  cmin[j][0:1, :])
            nc.sync.dma_start(
                cc_in[1:2, :].rearrange("o (p f) -> (o p) f", p=128), nrmin[:])
            nc.gpsimd.collective_compute(
                "AllReduce", A.max,
                replica_groups=[list(range(NCORES))],
                ins=[cc_in[:]], outs=[cc_out[:]],
            )

            # ---- post-collective scalar math in partition-major (128,64) ----
            gmr = small.tile([64, 128], f32)
            nc.sync.dma_start(
                gmr[:], cc_out[0:1, :].rearrange("o (f p) -> (o f) p", p=128))
            tg = psumT.tile([128, 64], f32, name="tg", tag="pst")
            nc.tensor.transpose(tg[:], gmr[:], ident[0:64, 0:64])
            gmaxP = small.tile([128, 64], f32)
            nc.scalar.copy(gmaxP[:], tg[:])
            nminP = small.tile([128, 64], f32)
            nc.sync.dma_start(
                nminP[:],
                cc_out[1:2, :].rearrange("o (p f) -> (o p) f", p=128))

            rng = small.tile([128, 64], f32)
            nc.vector.tensor_tensor(rng[:], gmaxP[:], nminP[:], op=A.add)
            eq0 = small.tile([128, 64], f32)
            nc.vector.tensor_scalar(out=eq0[:], in0=rng[:], scalar1=0.0,
                                    scalar2=None, op0=A.is_equal)
            safe = small.tile([128, 64], f32)
            nc.vector.tensor_tensor(safe[:], rng[:], eq0[:], op=A.add)
            rinv = small.tile([128, 64], f32)
            nc.vector.reciprocal(rinv[:], safe[:])
            minP = small.tile([128, 64], f32)
            nc.vector.tensor_scalar(out=minP[:], in0=nminP[:], scalar1=-1.0,
                                    scalar2=None, op0=A.mult)

            # relayout to natural rows via pack + PE transposes
            packa = small.tile([128, 128], f32)
            nc.vector.tensor_copy(packa[:, 0:64], minP[:])
            nc.vector.tensor_copy(packa[:, 64:128], rinv[:])
            ta = psumT.tile([128, 128], f32, name="ta", tag="pst")
            nc.tensor.transpose(ta[:], packa[:], ident[:])
            tas = small.tile([128, 128], f32)
            nc.scalar.copy(tas[:], ta[:])
            tb = psumT.tile([64, 128], f32, name="tb", tag="pst")
            nc.tensor.transpose(tb[:], safe[:], ident[:])
            tbs = small.tile([64, 128], f32)
            nc.scalar.copy(tbs[:], tb[:])

            mn_d = dram.tile([1, N], f32)
            rinv_d = dram.tile([1, N], f32)
            safe_d = dram.tile([1, N], f32)
            nc.sync.dma_start(
                mn_d[:].rearrange("o (f p) -> (o f) p", p=128), tas[0:64, :])
            nc.sync.dma_start(
                rinv_d[:].rearrange("o (f p) -> (o f) p", p=128),
                tas[64:128, :])
            nc.sync.dma_start(
                safe_d[:].rearrange("o (f p) -> (o f) p", p=128), tbs[:])

            # broadcasts: mnb in the cmax slots (lives to the end),
            # rb in the cmin slots (dies after startup; safeb reuses them)
            mnb = [mmq.tile([128, QW], f32, name=f"q{j}") for j in range(NQ)]
            rb = [mmq.tile([128, QW], f32, name=f"r{j}") for j in range(NQ)]
            for j in range(NQ):
                nc.sync.dma_start(
                    mnb[j][:],
                    mn_d[0:1, j * QW:(j + 1) * QW].to_broadcast((128, QW)))
                nc.sync.dma_start(
                    rb[j][:],
                    rinv_d[0:1, j * QW:(j + 1) * QW].to_broadcast((128, QW)))

            # ---- f16 DVE-slice state: tile 0 static, tiles 1-3 reuse the
            # rawH slots of the previous tile (fully read by then) ----
            arrD = [fd.tile([128, CD], f16, name="d0")] + \
                   [raw.tile([128, CD], f16, name=f"aH{t - 1}")
                    for t in range(1, NT)]

            sacc = small.tile([128, 8 * NT], f32)
            s0 = small.tile([128, NT], f32)
            h0 = small.tile([128, NT], f32)

            def startup(t):
                na = 0
                for j in range(NQ):
                    glo = j * QW
                    for tl, tlo, thi, g in rsegs(glo, glo + QW):
                        d0 = g - glo
                        w = thi - tlo
                        nc.vector.tensor_tensor(
                            tl[t][:, tlo:thi], tl[t][:, tlo:thi],
                            mnb[j][:, d0:d0 + w], op=A.subtract)
                for j in range(NQ):
                    glo = j * QW
                    for tl, tlo, thi, g in rsegs(glo, glo + QW):
                        d0 = g - glo
                        w = thi - tlo
                        dst = (rawL[t][:, tlo:thi] if tl is rawL
                               else arrD[t][:, tlo:thi])
                        nc.vector.scalar_tensor_tensor(
                            out=dst, in0=tl[t][:, tlo:thi], scalar=0.0,
                            in1=rb[j][:, d0:d0 + w], op0=A.bypass,
                            op1=A.mult,
                            accum_out=sacc[:, 8 * t + na:8 * t + na + 1])
                        na += 1
                nc.vector.tensor_reduce(
                    out=s0[:, t:t + 1], in_=sacc[:, 8 * t:8 * t + na],
                    axis=AX.X, op=A.add)
                nc.vector.tensor_scalar(
                    out=h0[:, t:t + 1], in0=s0[:, t:t + 1], scalar1=0.5 / N,
                    scalar2=0.5, op0=A.mult, op1=A.add)

            # per-group scalar-chain state (group g = tiles 2g, 2g+1)
            G = [dict() for _ in range(2)]

            def ginit(g):
                st = G[g]
                hv = small.tile([128, 2], f32, name=f"gh{g}")
                nc.vector.tensor_copy(hv[:], h0[:, 2 * g:2 * g + 2])
                nshv = small.tile([128, 2], f32, name=f"gnsh{g}")
                nc.vector.tensor_scalar(out=nshv[:], in0=hv[:], scalar1=-1.0,
                                        scalar2=None, op0=A.mult)
                hhv = small.tile([128, 2], f32, name=f"ghh{g}")
                nc.vector.tensor_tensor(hhv[:], hv[:], hv[:], op=A.mult)
                gamv = small.tile([128, 2], f32, name=f"ggam{g}")
                nc.vector.tensor_scalar(out=gamv[:], in0=hhv[:], scalar1=0.001,
                                        scalar2=None, op0=A.add)
                hgv = small.tile([128, 2], f32, name=f"ghg{g}")
                nc.vector.tensor_scalar(out=hgv[:], in0=gamv[:], scalar1=0.5,
                                        scalar2=0.5, op0=A.mult, op1=A.add)
                st.update(sh=hv, nsh=nshv, gam=gamv, hg=hgv)

            def giter(g, k):
                st = G[g]
                last = k == NITERS - 1
                accA = small.tile([128, 2], f32, name=f"gaccA{g}")
                accD = small.tile([128, 2], f32, name=f"gaccD{g}")
                for i, t in enumerate((2 * g, 2 * g + 1)):
                    nc.scalar.activation(
                        rawL[t][:], rawL[t][:], AF.Square,
                        bias=st["nsh"][:, i:i + 1], scale=1.0,
                        accum_out=accA[:, i:i + 1])
                    adv = arrD[t][:]
                    nc.vector.tensor_scalar(
                        out=adv, in0=adv, scalar1=st["sh"][:, i:i + 1],
                        scalar2=None, op0=A.subtract)
                    if last:
                        nc.vector.tensor_tensor(adv, adv, adv, op=A.mult)
                    else:
                        nc.vector.scalar_tensor_tensor(
                            out=adv, in0=adv, scalar=0.0, in1=adv,
                            op0=A.bypass, op1=A.mult,
                            accum_out=accD[:, i:i + 1])
                if last:
                    return
                ss = small.tile([128, 2], f32, name=f"gss{g}")
                nc.vector.tensor_tensor(ss[:], accA[:], accD[:], op=A.add)
                hv = small.tile([128, 2], f32, name=f"gh{g}")
                nc.vector.scalar_tensor_tensor(
                    out=hv[:], in0=ss[:], scalar=-0.5 / N, in1=st["hg"][:],
                    op0=A.mult, op1=A.add)
                beta = small.tile([128, 2], f32, name=f"gbeta{g}")
                nc.vector.tensor_tensor(beta[:], st["gam"][:], hv[:],
                                        op=A.subtract)
                nshv = small.tile([128, 2], f32, name=f"gnsh{g}")
                nc.vector.tensor_scalar(out=nshv[:], in0=beta[:], scalar1=-1.0,
                                        scalar2=None, op0=A.mult)
                gamv = small.tile([128, 2], f32, name=f"ggam{g}")
                nc.vector.tensor_tensor(gamv[:], hv[:], hv[:], op=A.mult)
                hgv = small.tile([128, 2], f32, name=f"ghg{g}")
                nc.vector.tensor_scalar(out=hgv[:], in0=gamv[:], scalar1=0.5,
                                        scalar2=0.5, op0=A.mult, op1=A.add)
                st.update(sh=beta, nsh=nshv, gam=gamv, hg=hgv)

            # safeb broadcast reuses the rb slots; issue before the pipeline
            # tail needs it (rb is fully read once all startups are issued,
            # so emit after the last startup in the schedule below).
            def bc_safeb():
                sb = [mmq.tile([128, QW], f32, name=f"r{j}")
                      for j in range(NQ)]
                for j in range(NQ):
                    nc.sync.dma_start(
                        sb[j][:],
                        safe_d[0:1, j * QW:(j + 1) * QW].to_broadcast(
                            (128, QW)))
                return sb

            def final(t):
                g = t // 2
                i = t % 2
                gamv = G[g]["gam"]
                outD = raw.tile([128, CD], f32, name="aH3")
                for j in range(NQ):
                    glo = j * QW
                    for tl, tlo, thi, gg in rsegs(glo, glo + QW):
                        d0 = gg - glo
                        w = thi - tlo
                        src = (rawL[t][:, tlo:thi] if tl is rawL
                               else arrD[t][:, tlo:thi])
                        nc.vector.scalar_tensor_tensor(
                            out=src, in0=src, scalar=gamv[:, i:i + 1],
                            in1=safeb[j][:, d0:d0 + w],
                            op0=A.subtract, op1=A.mult)
                for j in range(NQ):
                    glo = j * QW
                    for tl, tlo, thi, gg in rsegs(glo, glo + QW):
                        d0 = gg - glo
                        w = thi - tlo
                        src = (rawL[t][:, tlo:thi] if tl is rawL
                               else arrD[t][:, tlo:thi])
                        dst = (rawL[t][:, tlo:thi] if tl is rawL
                               else outD[:, tlo:thi])
                        nc.vector.tensor_tensor(
                            dst, mnb[j][:, d0:d0 + w], src, op=A.subtract)
                nc.sync.dma_start(ov[t][:, 0:CA], rawL[t][:])
                nc.sync.dma_start(ov[t][:, CA:N], outD[:])

            # ---- software pipeline: G0 iterates while G1 starts up ----
            startup(0)
            startup(1)
            ginit(0)
            giter(0, 0)
            startup(2)
            giter(0, 1)
            startup(3)
            safeb = bc_safeb()
            giter(0, 2)
            ginit(1)
            for k in range(3, NITERS):
                giter(0, k)
                giter(1, k - 3)
            final(0)
            giter(1, 7)
            final(1)
            giter(1, 8)
            giter(1, 9)
            final(2)
            final(3)

    if not nc.is_finalized():
        nc.finalize()
    return nc


def _get_nc():
    if "nc" not in _cache:
        _cache["nc"] = _build()
    return _cache["nc"]


def kernel(x):
    global LAST_RESULT
    from concourse.bass_utils import run_bass_kernel_spmd

    x = np.ascontiguousarray(np.asarray(x), dtype=np.float32)
    a = x.reshape(NCORES * R, N)
    nc = _get_nc()
    in_maps = [{"xs": np.ascontiguousarray(a[c * R:(c + 1) * R])}
               for c in range(NCORES)]
    res = run_bass_kernel_spmd(
        nc, in_maps, core_ids=list(range(NCORES)),
        trace=bool(int(os.environ.get("KBENCH_TRACE", "0"))),
    )
    LAST_RESULT = res
    full = np.concatenate([res.results[c]["out"] for c in range(NCORES)], axis=0)
    return full.reshape(1, NCORES * R, N).astype(np.float32)


# revision 57
# speedup vs baseline: 1.0874x; 1.0086x over previous
"""Trainium2 Bass kernel for nn_Net_76562087018570.

Computation (reference): per-column MinMax scale of a (4096, 8192) f32 matrix,
10 iterations of arr = arr*(1 - (arr - rowmean(arr))) (+0.001 on iter 0),
then inverse transform.

Strategy: shard rows across 8 cores (512 rows each, 4 x (128, 8192) tiles).
State carried as sq_k = (arr_k - h_k)^2 so each iteration is one fused
shift+square+row-sum pass, split between ACT (CA f32 cols, Square activation
with per-row bias and fused row-sum, in place in the load tile) and DVE
(CD f16 cols: TS-sub + TT-square + TS-accum, running at the 2x/4x DVE perf
modes). Startup ((a-mn)*rinv) and final (mn + safe*(gam-sq)) are plain
broadcast passes on DVE; the final result lands in the dead load-tile space
and DMAs out without staging copies. Column min/max: both combine chains on
DVE per 2048-quarter as tiles arrive, then gpsimd partition_all_reduce (max
side) + PE transposes/DVE reduces (min side), one packed AllReduce(max) on
[max | -min] (64 KiB). Broadcast vectors live in the minmax quarter slots.
"""

import os
import numpy as np

R = 512          # rows per core
N = 8192         # columns
NT = 4           # (128,N) row tiles per core
NQ = 4           # 2048-wide column quarters
QW = N // NQ
NCORES = 8
NITERS = 10
CA = 5504        # ACT slice (f32)
CD = N - CA      # 2688, DVE slice (f16)

_cache = {}
LAST_RESULT = None


def _build():
    import concourse.bacc as bacc
    import concourse.tile as tile
    from concourse import mybir, masks, bass_isa

    f32 = mybir.dt.float32
    f16 = mybir.dt.float16
    A = mybir.AluOpType
    AF = mybir.ActivationFunctionType
    AX = mybir.AxisListType

    nc = bacc.Bacc(trn_type="TRN2", num_devices=NCORES)
    xs = nc.dram_tensor("xs", [R, N], f32, kind="ExternalInput")
    out = nc.dram_tensor("out", [R, N], f32, kind="ExternalOutput")
    xv = xs.ap().rearrange("(t p) n -> t p n", p=128)
    ov = out.ap().rearrange("(t p) n -> t p n", p=128)

    with tile.TileContext(nc) as tc:
        with tc.tile_pool(name="raw", bufs=1) as raw, \
             tc.tile_pool(name="mmq", bufs=1) as mmq, \
             tc.tile_pool(name="fd", bufs=1) as fd, \
             tc.tile_pool(name="small", bufs=1) as small, \
             tc.tile_pool(name="psumT", bufs=2, space="PSUM") as psumT, \
             tc.tile_pool(name="dram", bufs=1, space="DRAM") as dram:

            ident = small.tile([128, 128], f32)
            masks.make_identity(nc, ident[:])

            # ---- load: row tile split at CA into rawL (f32, persists as the
            # ACT-slice state and output buffer) and rawH ----
            rawL = [raw.tile([128, CA], f32, name=f"aL{t}") for t in range(NT)]
            rawH = [raw.tile([128, CD], f32, name=f"aH{t}")
                    for t in range(NT)]
            for t in range(NT):
                nc.sync.dma_start(rawL[t][:], xv[t][:, 0:CA])
                nc.sync.dma_start(rawH[t][:], xv[t][:, CA:N])

            def rsegs(lo, hi):
                """Split [lo,hi) at CA between rawL/rawH; yield
                (tiles, tlo, thi, glo)."""
                o = []
                if lo < CA:
                    o.append((rawL, lo, min(hi, CA), lo))
                if hi > CA:
                    l = max(lo, CA)
                    o.append((rawH, l - CA, hi - CA, l))
                return o

            # ---- per-core column min/max per quarter, all on DVE ----
            cmax = [mmq.tile([128, QW], f32, name=f"q{j}") for j in range(NQ)]
            cmin = [mmq.tile([128, QW], f32, name=f"r{j}") for j in range(NQ)]
            for j in range(NQ):
                glo = j * QW
                pieces = rsegs(glo, glo + QW)
                for step in range(3):
                    for tl, tlo, thi, g in pieces:
                        d0, d1 = g - glo, g - glo + (thi - tlo)
                        if step == 0:
                            nc.vector.tensor_tensor(
                                cmax[j][:, d0:d1], tl[0][:, tlo:thi],
                                tl[1][:, tlo:thi], op=A.max)
                            nc.vector.tensor_tensor(
                                cmin[j][:, d0:d1], tl[0][:, tlo:thi],
                                tl[1][:, tlo:thi], op=A.min)
                        else:
                            nc.vector.tensor_tensor(
                                cmax[j][:, d0:d1], cmax[j][:, d0:d1],
                                tl[step + 1][:, tlo:thi], op=A.max)
                            nc.vector.tensor_tensor(
                                cmin[j][:, d0:d1], cmin[j][:, d0:d1],
                                tl[step + 1][:, tlo:thi], op=A.min)

            # min side: PE transposes + DVE min reduces -> partition-major
            # (128,64): rmin[p, j*16+cb] = min of col j*2048 + cb*128 + p.
            # max side: gpsimd partition_all_reduce into the cmin tile
            # (free after its transposes); row 0 = column max.
            rmin = small.tile([128, 64], f32)
            for j in range(NQ):
                for g in range(2):
                    pt = psumT.tile([128, 1024], f32, name="pt", tag="pst")
                    for b in range(8):
                        cb = g * 8 + b
                        nc.tensor.transpose(
                            pt[:, b * 128:(b + 1) * 128],
                            cmin[j][:, cb * 128:(cb + 1) * 128],
                            ident[:])
                    nc.vector.tensor_reduce(
                        out=rmin[:, j * 16 + g * 8:j * 16 + g * 8 + 8],
                        in_=pt[:].rearrange("p (c x) -> p c x", c=8),
                        axis=AX.X, op=A.min)
                nc.gpsimd.partition_all_reduce(
                    cmin[j][:], cmax[j][:], channels=128,
                    reduce_op=bass_isa.ReduceOp.max)
            nrmin = small.tile([128, 64], f32)
            nc.vector.tensor_scalar(out=nrmin[:], in0=rmin[:], scalar1=-1.0,
                                    scalar2=None, op0=A.mult)

            # ---- pack [gmax natural | -min partition-major], AllReduce(max)
            cc_in = dram.tile([2, 8192], f32)
            cc_out = dram.tile([2, 8192], f32, addr_space="Shared")
            for j in range(NQ):
                nc.sync.dma_start(cc_in[0:1, j * QW:(j + 1) * QW],
                                  cmin[j][0:1, :])
            nc.sync.dma_start(
                cc_in[1:2, :].rearrange("o (p f) -> (o p) f", p=128), nrmin[:])
            nc.gpsimd.collective_compute(
                "AllReduce", A.max,
                replica_groups=[list(range(NCORES))],
                ins=[cc_in[:]], outs=[cc_out[:]],
            )

            # ---- post-collective scalar math in partition-major (128,64) ----
            gmr = small.tile([64, 128], f32)
            nc.sync.dma_start(
                gmr[:], cc_out[0:1, :].rearrange("o (f p) -> (o f) p", p=128))
            tg = psumT.tile([128, 64], f32, name="tg", tag="pst")
            nc.tensor.transpose(tg[:], gmr[:], ident[0:64, 0:64])
            gmaxP = small.tile([128, 64], f32)
            nc.scalar.copy(gmaxP[:], tg[:])
            nminP = small.tile([128, 64], f32)
            nc.sync.dma_start(
                nminP[:],
                cc_out[1:2, :].rearrange("o (p f) -> (o p) f", p=128))

            rng = small.tile([128, 64], f32)
            nc.vector.tensor_tensor(rng[:], gmaxP[:], nminP[:], op=A.add)
            eq0 = small.tile([128, 64], f32)
            nc.vector.tensor_scalar(out=eq0[:], in0=rng[:], scalar1=0.0,
                                    scalar2=None, op0=A.is_equal)
            safe = small.tile([128, 64], f32)
            nc.vector.tensor_tensor(safe[:], rng[:], eq0[:], op=A.add)
            rinv = small.tile([128, 64], f32)
            nc.vector.reciprocal(rinv[:], safe[:])
            minP = small.tile([128, 64], f32)
            nc.vector.tensor_scalar(out=minP[:], in0=nminP[:], scalar1=-1.0,
                                    scalar2=None, op0=A.mult)

            # relayout to natural rows via pack + PE transposes
            packa = small.tile([128, 128], f32)
            nc.vector.tensor_copy(packa[:, 0:64], minP[:])
            nc.vector.tensor_copy(packa[:, 64:128], rinv[:])
            ta = psumT.tile([128, 128], f32, name="ta", tag="pst")
            nc.tensor.transpose(ta[:], packa[:], ident[:])
            tas = small.tile([128, 128], f32)
            nc.scalar.copy(tas[:], ta[:])
            tb = psumT.tile([64, 128], f32, name="tb", tag="pst")
            nc.tensor.transpose(tb[:], safe[:], ident[:])
            tbs = small.tile([64, 128], f32)
            nc.scalar.copy(tbs[:], tb[:])

            mn_d = dram.tile([1, N], f32)
            rinv_d = dram.tile([1, N], f32)
            safe_d = dram.tile([1, N], f32)
            nc.sync.dma_start(
                mn_d[:].rearrange("o (f p) -> (o f) p", p=128), tas[0:64, :])
            nc.sync.dma_start(
                rinv_d[:].rearrange("o (f p) -> (o f) p", p=128),
                tas[64:128, :])
            nc.sync.dma_start(
                safe_d[:].rearrange("o (f p) -> (o f) p", p=128), tbs[:])

            # broadcasts: mnb in the cmax slots (lives to the end),
            # rb in the cmin slots (dies after startup; safeb reuses them)
            mnb = [mmq.tile([128, QW], f32, name=f"q{j}") for j in range(NQ)]
            rb = [mmq.tile([128, QW], f32, name=f"r{j}") for j in range(NQ)]
            for j in range(NQ):
                nc.sync.dma_start(
                    mnb[j][:],
                    mn_d[0:1, j * QW:(j + 1) * QW].to_broadcast((128, QW)))
                nc.sync.dma_start(
                    rb[j][:],
                    rinv_d[0:1, j * QW:(j + 1) * QW].to_broadcast((128, QW)))

            # ---- f16 DVE-slice state: tile 0 static, tiles 1-3 reuse the
            # rawH slots of the previous tile (fully read by then) ----
            arrD = [fd.tile([128, CD], f16, name="d0")] + \
                   [raw.tile([128, CD], f16, name=f"aH{t - 1}")
                    for t in range(1, NT)]

            sacc = small.tile([128, 8 * NT], f32)
            s0 = small.tile([128, NT], f32)
            h0 = small.tile([128, NT], f32)

            def startup(t):
                na = 0
                for j in range(NQ):
                    glo = j * QW
                    for tl, tlo, thi, g in rsegs(glo, glo + QW):
                        d0 = g - glo
                        w = thi - tlo
                        nc.vector.tensor_tensor(
                            tl[t][:, tlo:thi], tl[t][:, tlo:thi],
                            mnb[j][:, d0:d0 + w], op=A.subtract)
                for j in range(NQ):
                    glo = j * QW
                    for tl, tlo, thi, g in rsegs(glo, glo + QW):
                        d0 = g - glo
                        w = thi - tlo
                        dst = (rawL[t][:, tlo:thi] if tl is rawL
                               else arrD[t][:, tlo:thi])
                        nc.vector.scalar_tensor_tensor(
                            out=dst, in0=tl[t][:, tlo:thi], scalar=0.0,
                            in1=rb[j][:, d0:d0 + w], op0=A.bypass,
                            op1=A.mult,
                            accum_out=sacc[:, 8 * t + na:8 * t + na + 1])
                        na += 1
                nc.vector.tensor_reduce(
                    out=s0[:, t:t + 1], in_=sacc[:, 8 * t:8 * t + na],
                    axis=AX.X, op=A.add)
                nc.vector.tensor_scalar(
                    out=h0[:, t:t + 1], in0=s0[:, t:t + 1], scalar1=0.5 / N,
                    scalar2=0.5, op0=A.mult, op1=A.add)

            # per-group scalar-chain state (group g = tiles 2g, 2g+1)
            G = [dict() for _ in range(2)]

            def ginit(g):
                st = G[g]
                hv = small.tile([128, 2], f32, name=f"gh{g}")
                nc.vector.tensor_copy(hv[:], h0[:, 2 * g:2 * g + 2])
                nshv = small.tile([128, 2], f32, name=f"gnsh{g}")
                nc.vector.tensor_scalar(out=nshv[:], in0=hv[:], scalar1=-1.0,
                                        scalar2=None, op0=A.mult)
                hhv = small.tile([128, 2], f32, name=f"ghh{g}")
                nc.vector.tensor_tensor(hhv[:], hv[:], hv[:], op=A.mult)
                gamv = small.tile([128, 2], f32, name=f"ggam{g}")
                nc.vector.tensor_scalar(out=gamv[:], in0=hhv[:], scalar1=0.001,
                                        scalar2=None, op0=A.add)
                hgv = small.tile([128, 2], f32, name=f"ghg{g}")
                nc.vector.tensor_scalar(out=hgv[:], in0=gamv[:], scalar1=0.5,
                                        scalar2=0.5, op0=A.mult, op1=A.add)
                st.update(sh=hv, nsh=nshv, gam=gamv, hg=hgv)

            def giter(g, k):
                st = G[g]
                last = k == NITERS - 1
                accA = small.tile([128, 2], f32, name=f"gaccA{g}")
                accD = small.tile([128, 2], f32, name=f"gaccD{g}")
                for i, t in enumerate((2 * g, 2 * g + 1)):
                    nc.scalar.activation(
                        rawL[t][:], rawL[t][:], AF.Square,
                        bias=st["nsh"][:, i:i + 1], scale=1.0,
                        accum_out=accA[:, i:i + 1])
                    adv = arrD[t][:]
                    nc.vector.tensor_scalar(
                        out=adv, in0=adv, scalar1=st["sh"][:, i:i + 1],
                        scalar2=None, op0=A.subtract)
                    if last:
                        nc.vector.tensor_tensor(adv, adv, adv, op=A.mult)
                    else:
                        nc.vector.scalar_tensor_tensor(
                            out=adv, in0=adv, scalar=0.0, in1=adv,
                            op0=A.bypass, op1=A.mult,
                            accum_out=accD[:, i:i + 1])
                if last:
                    return
                ss = small.tile([128, 2], f32, name=f"gss{g}")
                nc.vector.tensor_tensor(ss[:], accA[:], accD[:], op=A.add)
                hv = small.tile([128, 2], f32, name=f"gh{g}")
                nc.vector.scalar_tensor_tensor(
                    out=hv[:], in0=ss[:], scalar=-0.5 / N, in1=st["hg"][:],
                    op0=A.mult, op1=A.add)
                beta = small.tile([128, 2], f32, name=f"gbeta{g}")
                nc.vector.tensor_tensor(beta[:], st["gam"][:], hv[:],
                                        op=A.subtract)
                nshv = small.tile([128, 2], f32, name=f"gnsh{g}")
                nc.vector.tensor_scalar(out=nshv[:], in0=beta[:], scalar1=-1.0,
                                        scalar2=None, op0=A.mult)
                gamv = small.tile([128, 2], f32, name=f"ggam{g}")
                nc.vector.tensor_tensor(gamv[:], hv[:], hv[:], op=A.mult)
                hgv = small.tile([128, 2], f32, name=f"ghg{g}")
                nc.vector.tensor_scalar(out=hgv[:], in0=gamv[:], scalar1=0.5,
                                        scalar2=0.5, op0=A.mult, op1=A.add)
                st.update(sh=beta, nsh=nshv, gam=gamv, hg=hgv)

            # safeb broadcast reuses the rb slots; issue before the pipeline
            # tail needs it (rb is fully read once all startups are issued,
            # so emit after the last startup in the schedule below).
            def bc_safeb():
                sb = [mmq.tile([128, QW], f32, name=f"r{j}")
                      for j in range(NQ)]
                for j in range(NQ):
                    nc.sync.dma_start(
                        sb[j][:],
                        safe_d[0:1, j * QW:(j + 1) * QW].to_broadcast(
                            (128, QW)))
                return sb

            def final(t):
                g = t // 2
                i = t % 2
                gamv = G[g]["gam"]
                outD = raw.tile([128, CD], f32, name="aH3")
                for j in range(NQ):
                    glo = j * QW
                    for tl, tlo, thi, gg in rsegs(glo, glo + QW):
                        d0 = gg - glo
                        w = thi - tlo
                        src = (rawL[t][:, tlo:thi] if tl is rawL
                               else arrD[t][:, tlo:thi])
                        nc.vector.scalar_tensor_tensor(
                            out=src, in0=src, scalar=gamv[:, i:i + 1],
                            in1=safeb[j][:, d0:d0 + w],
                            op0=A.subtract, op1=A.mult)
                for j in range(NQ):
                    glo = j * QW
                    for tl, tlo, thi, gg in rsegs(glo, glo + QW):
                        d0 = gg - glo
                        w = thi - tlo
                        src = (rawL[t][:, tlo:thi] if tl is rawL
                               else arrD[t][:, tlo:thi])
                        dst = (rawL[t][:, tlo:thi] if tl is rawL
                               else outD[:, tlo:thi])
                        nc.vector.tensor_tensor(
                            dst, mnb[j][:, d0:d0 + w], src, op=A.subtract)
                nc.sync.dma_start(ov[t][:, 0:CA], rawL[t][:])
                nc.sync.dma_start(ov[t][:, CA:N], outD[:])

            # ---- software pipeline: G0 iterates while G1 starts up ----
            startup(0)
            startup(1)
            ginit(0)
            giter(0, 0)
            startup(2)
            giter(0, 1)
            startup(3)
            safeb = bc_safeb()
            giter(0, 2)
            ginit(1)
            for k in range(3, NITERS):
                giter(0, k)
                giter(1, k - 3)
            final(0)
            giter(1, 7)
            final(1)
            giter(1, 8)
            giter(1, 9)
            final(2)
            final(3)

    if not nc.is_finalized():
        nc.finalize()
    return nc


def _get_nc():
    if "nc" not in _cache:
        _cache["nc"] = _build()
    return _cache["nc"]


def kernel(x):
    global LAST_RESULT
    from concourse.bass_utils import run_bass_kernel_spmd

    x = np.ascontiguousarray(np.asarray(x), dtype=np.float32)
    a = x.reshape(NCORES * R, N)
    nc = _get_nc()
    in_maps = [{"xs": np.ascontiguousarray(a[c * R:(c + 1) * R])}
               for c in range(NCORES)]
    res = run_bass_kernel_spmd(
        nc, in_maps, core_ids=list(range(NCORES)),
        trace=bool(int(os.environ.get("KBENCH_TRACE", "0"))),
    )
    LAST_RESULT = res
    full = np.concatenate([res.results[c]["out"] for c in range(NCORES)], axis=0)
    return full.reshape(1, NCORES * R, N).astype(np.float32)
